# revision 42
# baseline (speedup 1.0000x reference)
"""Bahdanau attention weights kernel for 8 Trainium2 NeuronCores.

Reference computation (per full input):
    proj_enc = encoder_output @ W1_w + W1_b            # [B,S,U]
    proj_h   = last_layer_h_n @ W2_w + W2_b            # [B,1,U]
    score    = tanh(proj_enc + proj_h) @ V_w + V_b     # [B,S,1]
    out      = softmax(score, axis=1)                  # [B,S,1]

Sharding: data-parallel over batch. Each of the 8 cores gets B/8 batches;
weights are replicated; softmax is over the local sequence axis, so no
cross-core communication is needed.

Per-core layout strategy: keep U on partitions.
  - W1 [h,u] is the matmul stationary operand in its natural layout.
  - X^T tiles ([h, t]) DMA directly from the host-transposed bf16
    encoder copy.
  - Main matmuls compute proj^T [u=128, t=512] in PSUM, accumulated over
    8 h-blocks, in bf16 (inputs pre-rounded on host: identical numerics
    to an on-chip cast at half the DMA bytes).
  - tanh runs on the scalar engine reading PSUM, with the combined bias
    (W1_b + W2_b + h_n @ W2)[u] as the per-partition bias operand.
  - The V contraction runs on the DVE: acc += V_ub (.) tanh_ub with V as
    a per-partition f32 scalar; the last term writes bf16 directly and
    one all-ones matmul per group sums it over its 128 partitions.
  - Score rows live at partitions 0/32/64/96 (one per local batch); exp
    chunks accumulate per-batch sums and each batch's reduce/recip/mul/
    store chain runs as soon as its last group lands, hidden under later
    batches' compute (scores are bounded, so exp without max-subtraction
    is safe in fp32).

Schedule notes (from perfetto analysis of the 267us baseline):
  - ~8us of launch-barrier tax before any engine/DMA moves; warm-up
    matmuls on a memset tile ramp the PE p-state during the DMA window.
  - DMA issue order is X0 -> W1 -> hn -> W2: group-0 matmuls only need
    the first 3MB; the bias chain (W2) overlaps group-0 compute.
  - Group-0's first ub-chains are emitted BEFORE the bias matmuls on
    the PE queue; the bias result is only needed by group-0's tanh.
  - h_n / W2 use an h = p*8+hb partition mapping so the h_n^T DMA is
    128 contiguous 64B rows instead of 1024 8B descriptors (the bias
    contraction is invariant to the shared h permutation).
"""

import sys

for _p in ("/opt/trn_rl_repo", "/root/.axon_site/_ro/trn_rl_repo"):
    if _p not in sys.path:
        sys.path.append(_p)

import numpy as np

import concourse.bacc as bacc
import concourse.tile as tile
from concourse import mybir
from concourse.masks import make_identity

F32 = mybir.dt.float32
BF16 = mybir.dt.bfloat16

B, S, H, U = 32, 2048, 1024, 1024
N_CORES = 8
B_LOCAL = B // N_CORES  # 4
P = 128
T_GROUP = 512  # tokens per group (matmul moving dim)
N_WARM = 38    # PE warm-up matmuls (cover launch-window DMA + clock ramp)


def build_kernel(b_local=B_LOCAL, s=S, h=H, u=U):
    nc = bacc.Bacc()

    LP = BF16
    n_tok = b_local * s
    n_groups = n_tok // T_GROUP
    gpb = s // T_GROUP  # groups per batch
    HB = h // P
    UB = u // P
    UH = u // T_GROUP
    TSUB = T_GROUP // P

    # host supplies encoder_output and last_layer_h_n TRANSPOSED
    # ([h, tokens] / [h, b]) so X^T tiles DMA straight into SBUF
    enc = nc.dram_tensor("encoder_output", [h, n_tok], LP, kind="ExternalInput")
    hn = nc.dram_tensor("last_layer_h_n", [h, b_local], LP, kind="ExternalInput")
    w1 = nc.dram_tensor("W1_w", [h, u], LP, kind="ExternalInput")
    # b1/b2/V_w host-packed to [p, ub] (u = ub*128 + p) so their DMAs are
    # 128x32B rows instead of 1024 4-byte descriptors that clog the DMA
    # descriptor pipeline during the critical launch window.
    b1 = nc.dram_tensor("W1_b", [P, UB], F32, kind="ExternalInput")
    w2 = nc.dram_tensor("W2_w", [h, u], LP, kind="ExternalInput")
    b2 = nc.dram_tensor("W2_b", [P, UB], F32, kind="ExternalInput")
    # V_b is dropped entirely: a constant shift of every score in a row
    # is cancelled by the softmax.
    vw = nc.dram_tensor("V_w", [P, UB], F32, kind="ExternalInput")
    out = nc.dram_tensor("out", [b_local, s], F32, kind="ExternalOutput")

    encT_v = enc.ap().rearrange("(hb p) (g t) -> g p hb t", p=P, t=T_GROUP)
    encT4_v = enc.ap().rearrange(
        "(k j p) (g t) -> g k p j t", k=HB // 2, j=2, p=P, t=T_GROUP
    )
    w1_v = w1.ap().rearrange("(hb p) u -> hb p u", p=P)
    # h = p*8 + hb mapping for the bias contraction: h_n^T becomes one
    # contiguous 64B row per partition; W2 blocks use the same mapping.
    w2_v = w2.ap().rearrange("(p hb) u -> hb p u", hb=HB)
    hnT_v = hn.ap().rearrange("(p hb) b -> p hb b", hb=HB)

    NPREF = 5

    with tile.TileContext(nc) as tc:
        with (
            tc.tile_pool(name="consts", bufs=1) as consts,
            tc.tile_pool(name="wpool", bufs=1) as wpool,
            tc.tile_pool(name="xtpool", bufs=NPREF + 1) as xtpool,
            tc.tile_pool(name="thpool", bufs=4) as thpool,
            tc.tile_pool(name="accpool", bufs=3) as accpool,
            tc.tile_pool(name="scmpool", bufs=3) as scmpool,
            tc.tile_pool(name="rowpool", bufs=1) as rowpool,
            tc.tile_pool(name="psu", bufs=5, space="PSUM") as psu,
            tc.tile_pool(name="pst", bufs=2, space="PSUM") as pst,
            tc.tile_pool(name="psmg", bufs=1, space="PSUM") as psmg,
        ):
            # ---- PE warm-up on a memset tile: no DMA dependency, so it
            # starts right after the launch barrier and ramps the PE clock
            # while the first weight/X DMAs are in flight.
            warmT = consts.tile([P, T_GROUP], LP)
            nc.vector.memset(warmT, 0.5)
            wps = pst.tile([P, T_GROUP], F32, tag="tp")
            for i in range(N_WARM):
                nc.tensor.matmul(wps[:, :P], lhsT=warmT[:, :P], rhs=warmT[:, :P])

            ident = consts.tile([P, P], F32)
            make_identity(nc, ident)

            # ---- DMA issue order: tiny consts trickle on dynamic queues;
            # X0 + W1 gate the first real matmuls; hn + W2 (bias chain)
            # follow; then the X prefetch pipeline.
            v_sb = consts.tile([P, UB], F32)
            nc.sync.dma_start(out=v_sb, in_=vw.ap())
            b1_sb = consts.tile([P, UB], F32)
            nc.sync.dma_start(out=b1_sb, in_=b1.ap())
            b2_sb = consts.tile([P, UB], F32)
            nc.sync.dma_start(out=b2_sb, in_=b2.ap())
            x_pending = {}

            def issue_x(g):
                xT = xtpool.tile([P, HB, T_GROUP], LP, tag="xT")
                nc.sync.dma_start(out=xT, in_=encT_v[g])
                x_pending[g] = xT

            # group 0's X is split over 4 issues interleaved with the W1
            # blocks: each dma_start activates one hardware queue, so
            # spreading the first ~3MB over many issues ramps the
            # aggregate DMA rate much sooner.
            xT0 = xtpool.tile([P, HB, T_GROUP], LP, tag="xT")
            x_pending[0] = xT0
            w1_sb = []
            for hb in range(HB):
                t1 = wpool.tile([P, u], LP, tag=f"w1b_{hb}")
                w1_sb.append(t1)
            for k in range(HB // 2):
                nc.sync.dma_start(
                    out=xT0[:, 2 * k : 2 * k + 2, :], in_=encT4_v[0, k]
                )
                nc.sync.dma_start(out=w1_sb[2 * k], in_=w1_v[2 * k])
                nc.sync.dma_start(out=w1_sb[2 * k + 1], in_=w1_v[2 * k + 1])
            # hnT padded to 128 lhsT columns: the bias matmuls then run in
            # the same (128,128) PE tile config as the main matmuls — a
            # [128,4] lhsT would force (128,32) array reconfigs around the
            # whole bias phase. Rows 4-127 of the output are zero/unread.
            hnT = consts.tile([P, HB, P], LP)
            nc.vector.memset(hnT, 0.0)
            nc.sync.dma_start(out=hnT[:, :, :b_local], in_=hnT_v)
            w2_sb = []
            for hb in range(HB):
                t2 = wpool.tile([P, u], LP, tag=f"w2b_{hb}")
                nc.sync.dma_start(out=t2, in_=w2_v[hb])
                w2_sb.append(t2)
            for g0 in range(1, min(NPREF, n_groups)):
                issue_x(g0)

            b12_sb = consts.tile([P, UB], F32)
            nc.vector.tensor_add(b12_sb, b1_sb, b2_sb)

            # full 128-wide ones: the merge matmul then produces 128
            # identical score rows with the SAME (128,128) PE tile config
            # as the main matmuls — a [128,1] lhsT would force a (128,32)
            # array reconfig twice per group (~600ns each boundary).
            ones_sb = consts.tile([P, P], LP)
            nc.vector.memset(ones_sb, 1.0)

            bias_sb = consts.tile([P, UB, b_local], F32)

            def emit_bias():
                # bias[u, b] = h_n @ W2 + (b1 + b2): computed as [b, u]
                # with W2 as the 512-wide moving operand, then transposed
                # back to [u, b] blocks.
                for uh in range(UH):
                    ps4 = pst.tile([P, T_GROUP], F32, tag="tp")
                    for hb in range(HB):
                        nc.tensor.matmul(
                            ps4,
                            lhsT=hnT[:, hb, :],
                            rhs=w2_sb[hb][:, uh * T_GROUP : (uh + 1) * T_GROUP],
                            start=(hb == 0),
                            stop=(hb == HB - 1),
                        )
                    # full-width copy/transpose keep the (128,128) config;
                    # only rows/cols 0-3 carry data
                    bstage = thpool.tile([P, T_GROUP], F32, tag="bstage")
                    nc.vector.tensor_copy(bstage, ps4)
                    for i in range(TSUB):
                        ub = uh * TSUB + i
                        psb_t = pst.tile([P, T_GROUP], F32, tag="tp")
                        nc.tensor.transpose(
                            psb_t[:, :P],
                            bstage[:, i * P : (i + 1) * P],
                            ident,
                        )
                        nc.scalar.activation(
                            bias_sb[:, ub, :], psb_t[:, :b_local],
                            mybir.ActivationFunctionType.Identity,
                            bias=b12_sb[:, ub : ub + 1],
                        )

            # score rows at partitions 0/32/64/96: one per local batch
            sc_row = rowpool.tile([3 * 32 + 1, s], F32, name="sc_row")
            esums = rowpool.tile([3 * 32 + 1, gpb], F32, name="esums")

            esum = rowpool.tile([3 * 32 + 1, 1], F32, name="esum")
            rec = rowpool.tile([3 * 32 + 1, 1], F32, name="rec")

            state = {"scm": None, "pending": None}

            def finish_pe(scm, pb, pgi):
                score_ps = psmg.tile([P, T_GROUP], F32, tag="mg")
                nc.tensor.matmul(score_ps, lhsT=ones_sb, rhs=scm)
                row = score_ps[32 * pb : 32 * pb + 1, :]
                # score chunk -> exp incrementally. scores are bounded
                # (|score| <= sum|V_w| < 17), so exp without
                # max-subtraction is safe in fp32.
                nc.scalar.activation(
                    sc_row[32 * pb : 32 * pb + 1, pgi * T_GROUP : (pgi + 1) * T_GROUP],
                    row,
                    mybir.ActivationFunctionType.Exp,
                    accum_out=esums[32 * pb : 32 * pb + 1, pgi : pgi + 1],
                )
                if pgi == gpb - 1:
                    # normalize + store this batch's row; hidden under
                    # later batches' compute except for the last one
                    sl = slice(32 * pb, 32 * pb + 1)
                    nc.vector.tensor_reduce(
                        esum[sl, :], esums[sl, :],
                        axis=mybir.AxisListType.X, op=mybir.AluOpType.add,
                    )
                    nc.vector.reciprocal(rec[sl, :], esum[sl, :])
                    nc.vector.tensor_scalar_mul(
                        sc_row[sl, :], sc_row[sl, :], rec[sl, :]
                    )
                    nc.sync.dma_start(
                        out=out.ap()[pb : pb + 1, :], in_=sc_row[sl, :]
                    )

            for g in range(n_groups):
                b = g // gpb
                gi = g % gpb

                if g + NPREF < n_groups:
                    issue_x(g + NPREF)

                xT = x_pending.pop(g)

                # proj^T[u, t] blocks + tanh; the V contraction runs on
                # the DVE as acc += V_ub (.) tanh_ub (per-partition scalar);
                # the final term writes the bf16 merge operand directly.
                acc = accpool.tile([P, T_GROUP], F32, tag="acc")
                scm = scmpool.tile([P, T_GROUP], LP, tag="scm")

                def tanh_dve(ub, pu):
                    th = thpool.tile([P, T_GROUP], LP, tag="th")
                    nc.scalar.activation(
                        th, pu,
                        mybir.ActivationFunctionType.Tanh,
                        bias=bias_sb[:, ub, b : b + 1],
                    )
                    if ub == 0:
                        nc.vector.tensor_scalar_mul(acc, th, v_sb[:, 0:1])
                    else:
                        nc.vector.scalar_tensor_tensor(
                            scm if ub == UB - 1 else acc,
                            th, v_sb[:, ub : ub + 1], acc,
                            op0=mybir.AluOpType.mult,
                            op1=mybir.AluOpType.add,
                        )

                held = []
                for ub in range(UB):
                    pu = psu.tile([P, T_GROUP], F32, tag="pu")
                    for hb in range(HB):
                        nc.tensor.matmul(
                            pu,
                            lhsT=w1_sb[hb][:, ub * P : (ub + 1) * P],
                            rhs=xT[:, hb, :],
                            start=(hb == 0),
                            stop=(hb == HB - 1),
                        )
                    if g == 0 and ub < 3:
                        # group 0 runs ahead of the bias: hold the tanh
                        # (it reads bias_sb) until the bias is emitted
                        held.append((ub, pu))
                        continue
                    if g == 0 and ub == 3:
                        # W2 has landed during the first ub-chains; the PE
                        # computes the bias while tanh waits on it.
                        emit_bias()
                        for ub2, pu2 in held:
                            tanh_dve(ub2, pu2)
                        held = []
                    tanh_dve(ub, pu)
                    if ub == 3 and state["pending"] is not None:
                        # merge of the previous group lands here, after a
                        # full matmul chain has hidden the DVE tail
                        finish_pe(state["scm"], *state["pending"])
                        state["pending"] = None
                state["scm"] = scm
                state["pending"] = (b, gi)

            # flush the last group (includes the last batch's normalize)
            finish_pe(state["scm"], *state["pending"])

    nc.compile()
    return nc


def make_in_maps(inputs):
    """Shard the full inputs per core: big tensors pre-rounded to bf16,
    encoder_output / last_layer_h_n pre-transposed to [H, tokens] / [H, b]."""
    import ml_dtypes

    bf16 = ml_dtypes.bfloat16

    def f32(name):
        return np.ascontiguousarray(np.asarray(inputs[name], dtype=np.float32))

    def big(name):
        return f32(name).astype(bf16)

    enc = big("encoder_output")
    hn = big("last_layer_h_n")
    w1, w2 = big("W1_w"), big("W2_w")
    # [p, ub] packing (u = ub*128 + p) for 128-row DMA descriptors
    vw = np.ascontiguousarray(f32("V_w")[:, 0].reshape(U // 128, 128).T)
    b1 = np.ascontiguousarray(f32("W1_b").reshape(U // 128, 128).T)
    b2 = np.ascontiguousarray(f32("W2_b").reshape(U // 128, 128).T)

    in_maps = []
    for c in range(N_CORES):
        sl = slice(c * B_LOCAL, (c + 1) * B_LOCAL)
        e = enc[sl].reshape(B_LOCAL * S, H).T  # [H, tokens]
        n = hn[sl].T                           # [H, b]
        in_maps.append({
            "encoder_output": np.ascontiguousarray(e),
            "last_layer_h_n": np.ascontiguousarray(n),
            "W1_w": w1, "W1_b": b1, "W2_w": w2, "W2_b": b2,
            "V_w": vw,
        })
    return in_maps


def kernel(**inputs):
    from concourse.bass_utils import run_bass_kernel_spmd

    nc = build_kernel()
    in_maps = make_in_maps(inputs)
    res = run_bass_kernel_spmd(nc, in_maps, core_ids=list(range(N_CORES)))
    outs = [res.results[c]["out"].reshape(B_LOCAL, S, 1) for c in range(N_CORES)]
    return np.concatenate(outs, axis=0)


# revision 46
# speedup vs baseline: 1.0128x; 1.0128x over previous
"""Bahdanau attention weights kernel for 8 Trainium2 NeuronCores.

Reference computation (per full input):
    proj_enc = encoder_output @ W1_w + W1_b            # [B,S,U]
    proj_h   = last_layer_h_n @ W2_w + W2_b            # [B,1,U]
    score    = tanh(proj_enc + proj_h) @ V_w + V_b     # [B,S,1]
    out      = softmax(score, axis=1)                  # [B,S,1]

Sharding: data-parallel over batch. Each of the 8 cores gets B/8 batches;
weights are replicated; softmax is over the local sequence axis, so no
cross-core communication is needed.

Per-core layout strategy: keep U on partitions.
  - W1 [h,u] is the matmul stationary operand in its natural layout.
  - X^T tiles ([h, t]) DMA directly from the host-transposed bf16
    encoder copy.
  - Main matmuls compute proj^T [u=128, t=512] in PSUM, accumulated over
    8 h-blocks, in bf16 (inputs pre-rounded on host: identical numerics
    to an on-chip cast at half the DMA bytes).
  - tanh runs on the scalar engine reading PSUM, with the combined bias
    (W1_b + W2_b + h_n @ W2)[u] as the per-partition bias operand.
  - The V contraction runs on the DVE: acc += V_ub (.) tanh_ub with V as
    a per-partition f32 scalar; the last term writes bf16 directly and
    one all-ones matmul per group sums it over its 128 partitions.
  - Score rows live at partitions 0/32/64/96 (one per local batch); exp
    chunks accumulate per-batch sums and each batch's reduce/recip/mul/
    store chain runs as soon as its last group lands, hidden under later
    batches' compute (scores are bounded, so exp without max-subtraction
    is safe in fp32).

Schedule notes (from perfetto analysis of the 267us baseline):
  - ~8us of launch-barrier tax before any engine/DMA moves; warm-up
    matmuls on a memset tile ramp the PE p-state during the DMA window.
  - DMA issue order is X0 -> W1 -> hn -> W2: group-0 matmuls only need
    the first 3MB; the bias chain (W2) overlaps group-0 compute.
  - Group-0's first ub-chains are emitted BEFORE the bias matmuls on
    the PE queue; the bias result is only needed by group-0's tanh.
  - h_n / W2 use an h = p*8+hb partition mapping so the h_n^T DMA is
    128 contiguous 64B rows instead of 1024 8B descriptors (the bias
    contraction is invariant to the shared h permutation).
"""

import sys

for _p in ("/opt/trn_rl_repo", "/root/.axon_site/_ro/trn_rl_repo"):
    if _p not in sys.path:
        sys.path.append(_p)

import numpy as np

import concourse.bacc as bacc
import concourse.tile as tile
from concourse import mybir
from concourse.masks import make_identity

F32 = mybir.dt.float32
BF16 = mybir.dt.bfloat16

B, S, H, U = 32, 2048, 1024, 1024
N_CORES = 8
B_LOCAL = B // N_CORES  # 4
P = 128
T_GROUP = 512  # tokens per group (matmul moving dim)
N_WARM = 38    # PE warm-up matmuls (cover launch-window DMA + clock ramp)


def build_kernel(b_local=B_LOCAL, s=S, h=H, u=U):
    nc = bacc.Bacc()

    LP = BF16
    n_tok = b_local * s
    n_groups = n_tok // T_GROUP
    gpb = s // T_GROUP  # groups per batch
    HB = h // P
    UB = u // P
    UH = u // T_GROUP
    TSUB = T_GROUP // P

    # host supplies encoder_output and last_layer_h_n TRANSPOSED
    # ([h, tokens] / [h, b]) so X^T tiles DMA straight into SBUF
    enc = nc.dram_tensor("encoder_output", [h, n_tok], LP, kind="ExternalInput")
    hn = nc.dram_tensor("last_layer_h_n", [h, b_local], LP, kind="ExternalInput")
    w1 = nc.dram_tensor("W1_w", [h, u], LP, kind="ExternalInput")
    # b1/b2/V_w host-packed to [p, ub] (u = ub*128 + p) so their DMAs are
    # 128x32B rows instead of 1024 4-byte descriptors that clog the DMA
    # descriptor pipeline during the critical launch window.
    b1 = nc.dram_tensor("W1_b", [P, UB], F32, kind="ExternalInput")
    w2 = nc.dram_tensor("W2_w", [h, u], LP, kind="ExternalInput")
    b2 = nc.dram_tensor("W2_b", [P, UB], F32, kind="ExternalInput")
    # V_b is dropped entirely: a constant shift of every score in a row
    # is cancelled by the softmax.
    vw = nc.dram_tensor("V_w", [P, UB], F32, kind="ExternalInput")
    out = nc.dram_tensor("out", [b_local, s], F32, kind="ExternalOutput")

    encT_v = enc.ap().rearrange("(hb p) (g t) -> g p hb t", p=P, t=T_GROUP)
    encT4_v = enc.ap().rearrange(
        "(k j p) (g t) -> g k p j t", k=HB // 2, j=2, p=P, t=T_GROUP
    )
    w1_v = w1.ap().rearrange("(hb p) u -> hb p u", p=P)
    # h = p*8 + hb mapping for the bias contraction: h_n^T becomes one
    # contiguous 64B row per partition; W2 blocks use the same mapping.
    w2_v = w2.ap().rearrange("(p hb) u -> hb p u", hb=HB)
    hnT_v = hn.ap().rearrange("(p hb) b -> p hb b", hb=HB)

    NPREF = 5

    with tile.TileContext(nc) as tc:
        with (
            tc.tile_pool(name="consts", bufs=1) as consts,
            tc.tile_pool(name="wpool", bufs=1) as wpool,
            tc.tile_pool(name="xtpool", bufs=NPREF + 1) as xtpool,
            tc.tile_pool(name="thpool", bufs=4) as thpool,
            tc.tile_pool(name="accpool", bufs=3) as accpool,
            tc.tile_pool(name="scmpool", bufs=3) as scmpool,
            tc.tile_pool(name="rowpool", bufs=1) as rowpool,
            tc.tile_pool(name="psu", bufs=5, space="PSUM") as psu,
            tc.tile_pool(name="pst", bufs=2, space="PSUM") as pst,
            tc.tile_pool(name="psmg", bufs=1, space="PSUM") as psmg,
        ):
            # ---- PE warm-up on a memset tile: no DMA dependency, so it
            # starts right after the launch barrier and ramps the PE clock
            # while the first weight/X DMAs are in flight.
            warmT = consts.tile([P, T_GROUP], LP)
            nc.vector.memset(warmT, 0.5)
            wps = pst.tile([P, T_GROUP], F32, tag="tp")
            for i in range(N_WARM):
                nc.tensor.matmul(wps[:, :P], lhsT=warmT[:, :P], rhs=warmT[:, :P])

            ident = consts.tile([P, P], F32)
            make_identity(nc, ident)

            # ---- DMA issue order: tiny consts trickle on dynamic queues;
            # X0 + W1 gate the first real matmuls; hn + W2 (bias chain)
            # follow; then the X prefetch pipeline.
            # small consts go out on idle engines' dynamic DMA queues:
            # they bypass the Sync queue's launch stall AND free Sync
            # issue slots so X0/W1/W2 transfers enqueue sooner.
            v_sb = consts.tile([P, UB], F32)
            nc.scalar.dma_start(out=v_sb, in_=vw.ap())
            b1_sb = consts.tile([P, UB], F32)
            nc.scalar.dma_start(out=b1_sb, in_=b1.ap())
            b2_sb = consts.tile([P, UB], F32)
            nc.gpsimd.dma_start(out=b2_sb, in_=b2.ap())
            x_pending = {}

            def issue_x(g):
                xT = xtpool.tile([P, HB, T_GROUP], LP, tag="xT")
                nc.sync.dma_start(out=xT, in_=encT_v[g])
                x_pending[g] = xT

            # group 0's X is split over 4 issues interleaved with the W1
            # blocks: each dma_start activates one hardware queue, so
            # spreading the first ~3MB over many issues ramps the
            # aggregate DMA rate much sooner.
            xT0 = xtpool.tile([P, HB, T_GROUP], LP, tag="xT")
            x_pending[0] = xT0
            w1_sb = []
            for hb in range(HB):
                t1 = wpool.tile([P, u], LP, tag=f"w1b_{hb}")
                w1_sb.append(t1)
            for k in range(HB // 2):
                nc.sync.dma_start(
                    out=xT0[:, 2 * k : 2 * k + 2, :], in_=encT4_v[0, k]
                )
                nc.sync.dma_start(out=w1_sb[2 * k], in_=w1_v[2 * k])
                nc.sync.dma_start(out=w1_sb[2 * k + 1], in_=w1_v[2 * k + 1])
            hnT = consts.tile([P, HB, b_local], LP)
            nc.gpsimd.dma_start(out=hnT, in_=hnT_v)
            w2_sb = []
            for hb in range(HB):
                t2 = wpool.tile([P, u], LP, tag=f"w2b_{hb}")
                nc.sync.dma_start(out=t2, in_=w2_v[hb])
                w2_sb.append(t2)
            for g0 in range(1, min(NPREF, n_groups)):
                issue_x(g0)

            b12_sb = consts.tile([P, UB], F32)
            nc.vector.tensor_add(b12_sb, b1_sb, b2_sb)

            # full 128-wide ones: the merge matmul then produces 128
            # identical score rows with the SAME (128,128) PE tile config
            # as the main matmuls — a [128,1] lhsT would force a (128,32)
            # array reconfig twice per group (~600ns each boundary).
            ones_sb = consts.tile([P, P], LP)
            nc.vector.memset(ones_sb, 1.0)

            bias_sb = consts.tile([P, UB, b_local], F32)

            def emit_bias():
                # bias[u, b] = h_n @ W2 + (b1 + b2): computed as [b, u]
                # with W2 as the 512-wide moving operand, then transposed
                # back to [u, b] blocks.
                for uh in range(UH):
                    ps4 = pst.tile([P, T_GROUP], F32, tag="tp")
                    for hb in range(HB):
                        nc.tensor.matmul(
                            ps4[:b_local, :],
                            lhsT=hnT[:, hb, :],
                            rhs=w2_sb[hb][:, uh * T_GROUP : (uh + 1) * T_GROUP],
                            start=(hb == 0),
                            stop=(hb == HB - 1),
                        )
                    bstage = thpool.tile([b_local, T_GROUP], F32, tag="bstage")
                    nc.vector.tensor_copy(bstage, ps4[:b_local, :])
                    for i in range(TSUB):
                        ub = uh * TSUB + i
                        psb_t = pst.tile([P, T_GROUP], F32, tag="tp")
                        nc.tensor.transpose(
                            psb_t[:, :b_local],
                            bstage[:, i * P : (i + 1) * P],
                            ident[:b_local, :b_local],
                        )
                        nc.scalar.activation(
                            bias_sb[:, ub, :], psb_t[:, :b_local],
                            mybir.ActivationFunctionType.Identity,
                            bias=b12_sb[:, ub : ub + 1],
                        )

            # score rows at partitions 0/32/64/96: one per local batch
            sc_row = rowpool.tile([3 * 32 + 1, s], F32, name="sc_row")
            esums = rowpool.tile([3 * 32 + 1, gpb], F32, name="esums")

            esum = rowpool.tile([3 * 32 + 1, 1], F32, name="esum")
            rec = rowpool.tile([3 * 32 + 1, 1], F32, name="rec")

            state = {"scm": None, "pending": None}

            def finish_pe(scm, pb, pgi):
                score_ps = psmg.tile([P, T_GROUP], F32, tag="mg")
                nc.tensor.matmul(score_ps, lhsT=ones_sb, rhs=scm)
                row = score_ps[32 * pb : 32 * pb + 1, :]
                # score chunk -> exp incrementally. scores are bounded
                # (|score| <= sum|V_w| < 17), so exp without
                # max-subtraction is safe in fp32.
                nc.scalar.activation(
                    sc_row[32 * pb : 32 * pb + 1, pgi * T_GROUP : (pgi + 1) * T_GROUP],
                    row,
                    mybir.ActivationFunctionType.Exp,
                    accum_out=esums[32 * pb : 32 * pb + 1, pgi : pgi + 1],
                )
                if pgi == gpb - 1:
                    # normalize + store this batch's row; hidden under
                    # later batches' compute except for the last one
                    sl = slice(32 * pb, 32 * pb + 1)
                    nc.vector.tensor_reduce(
                        esum[sl, :], esums[sl, :],
                        axis=mybir.AxisListType.X, op=mybir.AluOpType.add,
                    )
                    nc.vector.reciprocal(rec[sl, :], esum[sl, :])
                    nc.vector.tensor_scalar_mul(
                        sc_row[sl, :], sc_row[sl, :], rec[sl, :]
                    )
                    nc.sync.dma_start(
                        out=out.ap()[pb : pb + 1, :], in_=sc_row[sl, :]
                    )

            for g in range(n_groups):
                b = g // gpb
                gi = g % gpb

                if g + NPREF < n_groups:
                    issue_x(g + NPREF)

                xT = x_pending.pop(g)

                # proj^T[u, t] blocks + tanh; the V contraction runs on
                # the DVE as acc += V_ub (.) tanh_ub (per-partition scalar);
                # the final term writes the bf16 merge operand directly.
                acc = accpool.tile([P, T_GROUP], F32, tag="acc")
                scm = scmpool.tile([P, T_GROUP], LP, tag="scm")

                def tanh_dve(ub, pu):
                    th = thpool.tile([P, T_GROUP], LP, tag="th")
                    nc.scalar.activation(
                        th, pu,
                        mybir.ActivationFunctionType.Tanh,
                        bias=bias_sb[:, ub, b : b + 1],
                    )
                    if ub == 0:
                        nc.vector.tensor_scalar_mul(acc, th, v_sb[:, 0:1])
                    else:
                        nc.vector.scalar_tensor_tensor(
                            scm if ub == UB - 1 else acc,
                            th, v_sb[:, ub : ub + 1], acc,
                            op0=mybir.AluOpType.mult,
                            op1=mybir.AluOpType.add,
                        )

                held = []
                for ub in range(UB):
                    pu = psu.tile([P, T_GROUP], F32, tag="pu")
                    for hb in range(HB):
                        nc.tensor.matmul(
                            pu,
                            lhsT=w1_sb[hb][:, ub * P : (ub + 1) * P],
                            rhs=xT[:, hb, :],
                            start=(hb == 0),
                            stop=(hb == HB - 1),
                        )
                    if g == 0 and ub < 3:
                        # group 0 runs ahead of the bias: hold the tanh
                        # (it reads bias_sb) until the bias is emitted
                        held.append((ub, pu))
                        continue
                    if g == 0 and ub == 3:
                        # W2 has landed during the first ub-chains; the PE
                        # computes the bias while tanh waits on it.
                        emit_bias()
                        for ub2, pu2 in held:
                            tanh_dve(ub2, pu2)
                        held = []
                    tanh_dve(ub, pu)
                    if ub == 3 and state["pending"] is not None:
                        # merge of the previous group lands here, after a
                        # full matmul chain has hidden the DVE tail
                        finish_pe(state["scm"], *state["pending"])
                        state["pending"] = None
                state["scm"] = scm
                state["pending"] = (b, gi)

            # flush the last group (includes the last batch's normalize)
            finish_pe(state["scm"], *state["pending"])

    nc.compile()
    return nc


def make_in_maps(inputs):
    """Shard the full inputs per core: big tensors pre-rounded to bf16,
    encoder_output / last_layer_h_n pre-transposed to [H, tokens] / [H, b]."""
    import ml_dtypes

    bf16 = ml_dtypes.bfloat16

    def f32(name):
        return np.ascontiguousarray(np.asarray(inputs[name], dtype=np.float32))

    def big(name):
        return f32(name).astype(bf16)

    enc = big("encoder_output")
    hn = big("last_layer_h_n")
    w1, w2 = big("W1_w"), big("W2_w")
    # [p, ub] packing (u = ub*128 + p) for 128-row DMA descriptors
    vw = np.ascontiguousarray(f32("V_w")[:, 0].reshape(U // 128, 128).T)
    b1 = np.ascontiguousarray(f32("W1_b").reshape(U // 128, 128).T)
    b2 = np.ascontiguousarray(f32("W2_b").reshape(U // 128, 128).T)

    in_maps = []
    for c in range(N_CORES):
        sl = slice(c * B_LOCAL, (c + 1) * B_LOCAL)
        e = enc[sl].reshape(B_LOCAL * S, H).T  # [H, tokens]
        n = hn[sl].T                           # [H, b]
        in_maps.append({
            "encoder_output": np.ascontiguousarray(e),
            "last_layer_h_n": np.ascontiguousarray(n),
            "W1_w": w1, "W1_b": b1, "W2_w": w2, "W2_b": b2,
            "V_w": vw,
        })
    return in_maps


def kernel(**inputs):
    from concourse.bass_utils import run_bass_kernel_spmd

    nc = build_kernel()
    in_maps = make_in_maps(inputs)
    res = run_bass_kernel_spmd(nc, in_maps, core_ids=list(range(N_CORES)))
    outs = [res.results[c]["out"].reshape(B_LOCAL, S, 1) for c in range(N_CORES)]
    return np.concatenate(outs, axis=0)
